# revision 40
# baseline (speedup 1.0000x reference)
"""Trainium2 Bass kernel for nn_EternalRecursion (GRUCell self-recursion, B=512, D=500).

Strategy
--------
Data-parallel over 8 NeuronCores: 64 batch rows per core, GRU weights replicated.

Math restructuring (host-side, exact):
  - After step 1 the reference feeds h_new as BOTH x and h of the GRU cell, so
    steps >= 2 use combined weights W_rz = (W_ih+W_hh)[0:1000] for the r/z gates,
    while the n-gate keeps W_ih_n / W_hh_n separate (r multiplies only the h-side).
  - Step 1 (x=state, h=0) uses W_ih with a zero block for the h-side n columns,
    which makes it the *same* device code path with different weights.
  - Biases are folded into the matmul via an extra contraction row of ones.
  - The break check "mean(h_k) > bc" latches the output at the first step k*
    whose global mean exceeds bc. The device free-runs L steps, records per-step
    per-partition sums (free side-output of the last fused DVE op), and the host
    computes the global means. If the break fires before the last step (it cannot
    for the harness inputs: |h|<1 and bc>=0.9 keeps means far below bc), the
    kernel is re-built with L=k* and re-run, which reproduces the latched output.

Device layout (per core, per step):
  - h is stored "packed": [128 partitions, 250 free] with partition 64*H+b
    holding h[b, 250*H + c]. All elementwise gate math runs on [128, 250] tiles.
  - Gate pre-activations go into two [128, 512] PSUM tiles: region 1 = [gr | gz]
    (packed: partition half H holds gate columns 250H:250H+250), region 2 =
    [gin | ghn]. Each region is computed by 4 K-chunk slots; each slot is a PAIR
    of concurrent column-tiled matmuls [K<=126, M=64, N=500]: tile A
    (tile_position (0,0)) produces PSUM partitions 0:64 (H=0 gate columns),
    tile B ((0,64)) partitions 64:128 (H=1 columns). The two tiles share the
    same stationary (h^T K-chunk, batch in array columns) and stream different
    weight blocks, so the pair streams in ~one matmul's time. This replaces the
    previous K-doubled M=128 scheme (2x redundant MACs).
  - The stationary h^T lives in a [126, 256] SBUF tile: K-chunk g at columns
    64g:64g+64, with h-dim blocks g0=0:125, g1=250:375, g2=125:250, g3=375:500
    (chosen so PE-transposing hnew[:, 0:125] yields chunks g0|g1 side by side
    and hnew[:, 125:250] yields g2|g3). Row 125 of chunk g1 holds the ones row
    for the bias fold (the matching weight blocks carry the biases).
  - The gate chain is split in two 125-column chunks; chunk 0's chain feeds
    transpose A early so the next step's first matmul pairs can start while
    chunk 1 is still in the pipes. SBUF-only elementwise ops (zm1, zh, t2) run
    on GPSIMD (no PSUM port there), PSUM-reading ops on DVE, activations and
    one transpose-copy per half on ACT.
"""

import os
import sys
import types
import numpy as np
import ml_dtypes

BF16NP = ml_dtypes.bfloat16

D = 500
B = 512
NCORES = 8
BS = B // NCORES          # 64 batch rows per core
HALF = 250                # free columns of the packed layout
# h-dim blocks of the 4 K-chunks (order matches the transpose outputs)
CHUNK_DIMS = ((0, 125), (250, 375), (125, 250), (375, 500))
KG = (125, 126, 125, 125)  # chunk g1 carries the ones/bias row


def _install_hook_module():
    """Provide antenv.axon_hooks (missing from the RO image) so NTFF tracing
    through bass_utils can work when requested. Harmless if anything fails."""
    if "antenv.axon_hooks" in sys.modules:
        return
    mod = types.ModuleType("antenv.axon_hooks")
    holder = [None]
    mod.set_axon_ntff_profile_hook = lambda h: holder.__setitem__(0, h)
    mod.get_axon_ntff_profile_hook = lambda: holder[0]
    sys.modules["antenv.axon_hooks"] = mod
    try:
        from trn_agent_boot.trn_boot import _ntff_profile_via_ctypes
        hook = _ntff_profile_via_ctypes("/opt/axon/libaxon_pjrt.so")
        mod.set_axon_ntff_profile_hook(hook)
    except Exception:
        pass


_install_hook_module()

import concourse.bass as bass  # noqa: E402
import concourse.mybir as mybir  # noqa: E402
import concourse.tile as tile  # noqa: E402
from concourse import bass_utils  # noqa: E402
from concourse.masks import make_identity  # noqa: E402
import bass_rust  # noqa: E402

F32 = mybir.dt.float32
F32R = mybir.dt.float32r
BF16 = mybir.dt.bfloat16
AF = mybir.ActivationFunctionType
ALU = mybir.AluOpType


def _split_overwide_waits(nc, maxw=1):
    """walrus here rejects >1 sync wait per instruction; spread extras over
    preceding NoOp carriers. Most multi-wait instructions get same-engine
    carriers (order-preserving); the kernel-end drain (many loose-end waits)
    gets carriers round-robined across all engines so they resolve in
    parallel before the final barrier instead of serially on one engine."""
    n_new = 0
    all_engines = (mybir.EngineType.SP, mybir.EngineType.Activation,
                   mybir.EngineType.PE, mybir.EngineType.DVE,
                   mybir.EngineType.Pool)
    for fn in nc.m.functions:
        for bb in fn.blocks:
            out = []
            for inst in bb.instructions:
                si = inst.sync_info
                if si is not None and si.on_wait and len(si.on_wait) > maxw:
                    waits = list(si.on_wait)
                    chunks = [waits[i:i + maxw] for i in range(0, len(waits), maxw)]
                    spread = len(chunks) > 4  # only the big end-of-kernel drain
                    for j, ch in enumerate(chunks[:-1]):
                        eng = all_engines[j % len(all_engines)] if spread \
                            else inst.engine
                        nd = mybir.InstNoOp(
                            name=f"I-swx{n_new}", engine=eng,
                            bass_nofuse=True,
                            sync_info=bass_rust.SyncInfo(on_wait=ch, on_update=[]))
                        n_new += 1
                        nc.register_instruction(nd, overwrite=True)
                        out.append(nd)
                    inst.sync_info = bass_rust.SyncInfo(
                        on_wait=chunks[-1], on_update=list(si.on_update or []))
                out.append(inst)
            bb.instructions = out
    return n_new


def _build(L):
    """Build the Bass module for L GRU steps. Returns nc."""
    assert L >= 1
    nc = bass.Bass("TRN2", target_bir_lowering=False, debug=False)

    statet_d = nc.dram_tensor("statet", [126, 256], BF16, kind="ExternalInput").ap()
    wa_d = nc.dram_tensor("wa", [2, 126, 4000], BF16, kind="ExternalInput").ap()
    wb_d = nc.dram_tensor("wb", [2, 126, 4000], BF16, kind="ExternalInput").ap()
    hout_d = nc.dram_tensor("hout", [128, HALF], F32R, kind="ExternalOutput").ap()
    sums_d = nc.dram_tensor("sums", [128, 2 * L], F32, kind="ExternalOutput").ap()

    with tile.TileContext(nc) as tc:
        import contextlib
        with contextlib.ExitStack() as ctx:
            consts = ctx.enter_context(tc.tile_pool(name="consts", bufs=1))
            wpool = ctx.enter_context(tc.tile_pool(name="weights", bufs=1))
            hpool = ctx.enter_context(tc.tile_pool(name="hstate", bufs=1))
            work = ctx.enter_context(tc.tile_pool(name="work", bufs=2))
            gpsum = ctx.enter_context(tc.tile_pool(name="gpsum", bufs=2, space="PSUM"))
            tpsum = ctx.enter_context(tc.tile_pool(name="tpsum", bufs=2, space="PSUM"))

            identity = consts.tile([128, 128], F32, tag="identity", name="identity")
            make_identity(nc, identity[:])
            # f32r view for the transposes (verifier wants an f32r producer)
            identity_r = consts.tile([128, 128], F32R, tag="identity_r", name="identity_r")
            nc.vector.tensor_copy(identity_r[:], identity[:])

            statet = wpool.tile([126, 256], BF16, tag="statet", name="statet")
            nc.gpsimd.dma_start(statet[:], statet_d)
            # fused weight loads: 4 large DMAs instead of 16 (the ~1-2 us
            # per-DMA issue overhead dominated kernel startup)
            wa_t = [wpool.tile([126, 4000], BF16, tag=f"wah{h}", name=f"wah{h}")
                    for h in range(2)]
            wb_t = [wpool.tile([126, 4000], BF16, tag=f"wbh{h}", name=f"wbh{h}")
                    for h in range(2)]
            nc.gpsimd.dma_start(wa_t[0][:], wa_d[0])
            nc.sync.dma_start(wa_t[1][:], wa_d[1])
            nc.gpsimd.dma_start(wb_t[0][:], wb_d[0])
            nc.sync.dma_start(wb_t[1][:], wb_d[1])

            hT = [hpool.tile([126, 256], BF16, tag="hta", name="hta"),
                  hpool.tile([126, 256], BF16, tag="htb", name="htb")]
            # only row 125 needs init (ones at chunk g1, zeros elsewhere);
            # rows 0:125 are fully overwritten by the transpose copies before
            # first use. DVE ops can't start at partition 125, DMA can.
            nc.gpsimd.dma_start(hT[0][125:126, :], statet_d[125:126, :])
            nc.gpsimd.dma_start(hT[1][125:126, :], statet_d[125:126, :])

            sums = consts.tile([128, 2 * L], F32, tag="sums", name="sums")

            hprev = None  # packed [128, 250] h of the previous step
            hnew = None
            for k in range(1, L + 1):
                first = k == 1
                lhs_tile = statet if first else hT[k % 2]
                wt = wa_t if first else wb_t

                # separate PSUM tiles per bank so the rz consumers don't
                # wait on the n-block matmuls
                grz = gpsum.tile([128, 512], F32, tag="grz", name="grz")
                gn = gpsum.tile([128, 512], F32, tag="gn", name="gn")

                def mm_slot(out_tile, col0, g, off_base, width):
                    kg = KG[g]
                    lhsT = lhs_tile[0:kg, 64 * g:64 * g + 64]
                    for H in (0, 1):
                        off = (g % 2) * 2000 + off_base + H * width
                        nc.tensor.matmul(
                            out_tile[64 * H:64 * H + 64, col0:col0 + width],
                            lhsT, wt[g // 2][0:kg, off:off + width],
                            start=(g == 0), stop=(g == 3),
                            tile_position=(0, 64 * H),
                            skip_group_check=True)

                # 12 pair-slots: rz g0-3 (N=500), then the n-region split in
                # column halves (N=250 each) so chunk 0's chain starts 4 slots
                # earlier than a monolithic n region would allow
                for g in range(4):
                    mm_slot(grz, 0, g, 0, 500)
                for g in range(4):
                    mm_slot(gn, 0, g, 1000, 250)
                for g in range(4):
                    mm_slot(gn, 250, g, 1500, 250)

                rz = work.tile([128, 2 * HALF], F32, tag="rz", name="rz")
                # r = sigmoid(gr); w = 1-z = sigmoid(-gz) straight from PSUM.
                # z itself is never materialized: z*h = h - w*h.
                nc.scalar.activation(rz[:, 0:250], grz[:, 0:250], AF.Sigmoid)
                r = rz[:, 0:250]
                zm1 = work.tile([128, HALF], F32, tag="zm1", name="zm1")
                nc.scalar.activation(zm1[:], grz[:, 250:500], AF.Sigmoid,
                                     scale=-1.0)

                rhn = work.tile([128, HALF], F32R, tag="rhn", name="rhn")
                targ = work.tile([128, HALF], F32R, tag="targ", name="targ")
                n = work.tile([128, HALF], F32R, tag="n", name="n")
                t2 = work.tile([128, HALF], F32R, tag="t2", name="t2")
                hnew = work.tile([128, HALF], F32R, tag="hnew", name="hnew")
                if k < L:
                    dst = hT[(k + 1) % 2]

                # PSUM-reading chain ops on DVE, chunk 0 first
                # (n-region PSUM layout is [gin0|ghn0|gin1|ghn1], 125 each)
                for c in (0, 1):
                    s = slice(125 * c, 125 * (c + 1))
                    nc.vector.tensor_mul(rhn[:, s], r[:, s],
                                         gn[:, 250 * c + 125:250 * c + 250])
                    nc.vector.tensor_add(targ[:, s], rhn[:, s],
                                         gn[:, 250 * c:250 * c + 125])
                nc.scalar.activation(n[:, 0:125], targ[:, 0:125], AF.Tanh)
                nc.scalar.activation(n[:, 125:250], targ[:, 125:250], AF.Tanh)

                # Dummy matmul chain through the serial tail: the HAM clock
                # gate re-throttles the PE to 1.2 GHz whenever an activity
                # window has significant idle time; once cold, the throttle
                # phase-locks against the matmul stream (stream cold, tail
                # warm). The dummies are UNANCHORED (ready at step start, so
                # zero start lag) and WAW-serialized through dmy, tiling the
                # tail back-to-back from the moment the stream ends. f32r
                # with N<256 runs at quarter rate -> ~420ns each at any clock.
                if k < L:
                    dmy = tpsum.tile([1, 512], F32, tag="dmy", name="dmy", bufs=1)

                def dummy(count=1):
                    # f32r M=1 dummies: ~115ns of PE-busy each at full clock,
                    # ~210ns cold, negligible power (one array column active;
                    # fp32-mode dummies here trip the chip-level P0 power
                    # throttle, downclocking every engine ~20%)
                    if k >= L or hprev is None:
                        return
                    for _ in range(count):
                        nc.tensor.matmul(dmy[:, 0:250], hprev[0:128, 0:1],
                                         hprev[0:128, 0:250])

                if k < L:
                    nc.tensor.matmul(dmy[:], lhs_tile[0:126, 0:1],
                                     wt[0][0:126, 0:512])
                    dummy(16)

                # zh = z*h = h - w*h as two pure-TensorTensor GPSIMD ops
                # (all-SBUF, off the critical path; Pool has no PSUM port and
                # no scalar-operand opcodes; keeps ACT at 2 sigmoids and DVE
                # at the chain)
                wh = work.tile([128, HALF], F32, tag="wh", name="wh")
                zh = work.tile([128, HALF], F32, tag="zh", name="zh")
                if not first:
                    # chunked so zh[:, 0:125] is ready before hnew chunk 0
                    for c in (0, 1):
                        s = slice(125 * c, 125 * (c + 1))
                        nc.gpsimd.tensor_mul(wh[:, s], zm1[:, s], hprev[:, s])
                        nc.gpsimd.tensor_sub(zh[:, s], hprev[:, s], wh[:, s])

                pT = [None, None]
                for c in (0, 1):
                    s = slice(125 * c, 125 * (c + 1))
                    acc = sums[:, 2 * (k - 1) + c:2 * (k - 1) + c + 1]
                    if first:
                        # h == 0: h_new = n * (1 - z)
                        nc.vector.scalar_tensor_tensor(
                            hnew[:, s], n[:, s], 0.0, zm1[:, s],
                            op0=ALU.bypass, op1=ALU.mult, accum_out=acc)
                    else:
                        nc.vector.tensor_mul(t2[:, s], n[:, s], zm1[:, s])
                        nc.vector.scalar_tensor_tensor(
                            hnew[:, s], t2[:, s], 0.0, zh[:, s],
                            op0=ALU.bypass, op1=ALU.add, accum_out=acc)
                    if k < L:
                        # transpose via a REGULAR matmul (in.T @ I): real PE
                        # activity for the HAM clock gate, and PSUM output
                        pT[c] = tpsum.tile([125, 128], F32, tag="pT",
                                           name=f"p{'AB'[c]}")
                        nc.tensor.matmul(pT[c][:], hnew[:, s], identity_r[:])
                        if c == 0:
                            dummy(2)
                if k < L:
                    # PSUM -> SBUF stationary copies: transpose A as one
                    # [125, 128] ACT copy; transpose B split ACT+DVE halves
                    # (its consumers are the last stream slots)
                    nc.scalar.copy(dst[0:125, 0:128], pT[0][:])
                    nc.scalar.copy(dst[0:125, 128:192], pT[1][:, 0:64])
                    nc.vector.tensor_copy(dst[0:125, 192:256], pT[1][:, 64:128])
                hprev = hnew

            nc.gpsimd.dma_start(hout_d, hnew[:])
            nc.gpsimd.dma_start(sums_d, sums[:])

    _split_overwide_waits(nc)
    return nc


_NC_CACHE = {}


def _get_nc(L):
    if L not in _NC_CACHE:
        _NC_CACHE[L] = _build(L)
    return _NC_CACHE[L]


def _prep_weights(W_ih, W_hh, b_ih, b_hh):
    """Build wa/wb DRAM images [2, 126, 4000].

    Column layout per K-chunk g (base (g%2)*2000):
      +0    : RZ block,  H0: [r(250)|z(250)]      +500  : H1 same
      +1000 : N0 block,  H0: [gin-c0(125)|ghn-c0(125)]   +1250: H1
      +1500 : N1 block,  H0: [gin-c1|ghn-c1]             +1750: H1
    Rows 0:125 = W^T rows for the chunk's h-dims; row 125 = bias
    (nonzero only for chunk g1)."""
    W_ih = np.asarray(W_ih, np.float32)
    W_hh = np.asarray(W_hh, np.float32)
    b_ih = np.asarray(b_ih, np.float32)
    b_hh = np.asarray(b_hh, np.float32)

    br = b_ih[0:500] + b_hh[0:500]
    bz = b_ih[500:1000] + b_hh[500:1000]
    bin_ = b_ih[1000:1500]
    bhn = b_hh[1000:1500]

    def put(img, g, off, left, right, bias):
        w = left.shape[0]
        img[g // 2, 0:125, off:off + 2 * w] = np.hstack([left.T, right.T])
        if g == 1:
            img[g // 2, 125, off:off + 2 * w] = bias

    def pack(Wrz, Win, Whn):
        img = np.zeros((2, 126, 4000), np.float32)
        for g in range(4):
            d0, d1 = CHUNK_DIMS[g]
            base = (g % 2) * 2000
            for H in range(2):
                lo = 250 * H
                # RZ block (500 wide per half)
                put(img, g, base + 500 * H,
                    Wrz[lo:lo + 250, d0:d1], Wrz[500 + lo:500 + lo + 250, d0:d1],
                    np.concatenate([br[lo:lo + 250], bz[lo:lo + 250]]))
                # N0 / N1 blocks (250 wide per half, chain-chunk column split)
                for cb in range(2):
                    cl = lo + 125 * cb
                    put(img, g, base + 1000 + 500 * cb + 250 * H,
                        Win[cl:cl + 125, d0:d1], Whn[cl:cl + 125, d0:d1],
                        np.concatenate([bin_[cl:cl + 125], bhn[cl:cl + 125]]))
        return np.ascontiguousarray(img.astype(BF16NP))

    zeros = np.zeros((500, D), np.float32)
    WB = pack(W_ih[0:1000] + W_hh[0:1000], W_ih[1000:1500], W_hh[1000:1500])
    WA = pack(W_ih[0:1000], W_ih[1000:1500], zeros)
    return WA, WB


def _prep_state(state):
    """Per-core stationary state^T images [126, 256] (chunk g at cols 64g)."""
    state = np.asarray(state, np.float32)
    outs = []
    for c in range(NCORES):
        shard = state[BS * c:BS * (c + 1)]            # [64, 500]
        img = np.zeros((126, 256), np.float32)
        for g in range(4):
            d0, d1 = CHUNK_DIMS[g]
            img[0:125, 64 * g:64 * g + 64] = shard[:, d0:d1].T
        img[125, 64:128] = 1.0                        # ones row of chunk g1
        outs.append(img.astype(BF16NP))
    return outs


def _run(L, stateTs, wa, wb, trace=False):
    nc = _get_nc(L)
    in_maps = [{"statet": np.ascontiguousarray(stateTs[c]),
                "wa": wa, "wb": wb} for c in range(NCORES)]
    res = bass_utils.run_bass_kernel_spmd(
        nc, in_maps, core_ids=list(range(NCORES)), trace=trace)
    shards = []
    sums = np.zeros((128, 2 * L), np.float64)
    for c in range(NCORES):
        hout = res.results[c]["hout"]
        shards.append(np.concatenate([hout[0:64], hout[64:128]], axis=1))
        sums += res.results[c]["sums"].astype(np.float64)
    h = np.concatenate(shards, axis=0)                # [512, 500]
    means = (sums[:, 0::2] + sums[:, 1::2]).sum(axis=0) / (B * D)  # [L]
    return h, means, res


def kernel(state, W_ih, W_hh, b_ih, b_hh, break_condition, recursion_limit):
    state = np.asarray(state, np.float32)
    L = int(np.asarray(recursion_limit))
    if L <= 0:
        return state.copy()
    bc = float(np.asarray(break_condition))

    wa, wb = _prep_weights(W_ih, W_hh, b_ih, b_hh)
    stateTs = _prep_state(state)

    h, means, _ = _run(L, stateTs, wa, wb)
    fired = np.nonzero(means > bc)[0]
    if fired.size and fired[0] + 1 < L:
        # break fired at step k* = fired[0]+1: output latches h_{k*}
        h, _, _ = _run(int(fired[0]) + 1, stateTs, wa, wb)
    return h.astype(np.float32)


# revision 41
# speedup vs baseline: 1.1866x; 1.1866x over previous
"""Trainium2 Bass kernel for nn_EternalRecursion (GRUCell self-recursion, B=512, D=500).

Strategy
--------
Data-parallel over 8 NeuronCores: 64 batch rows per core, GRU weights replicated.

Math restructuring (host-side, exact):
  - After step 1 the reference feeds h_new as BOTH x and h of the GRU cell, so
    steps >= 2 use combined weights W_rz = (W_ih+W_hh)[0:1000] for the r/z gates,
    while the n-gate keeps W_ih_n / W_hh_n separate (r multiplies only the h-side).
  - Step 1 (x=state, h=0) uses W_ih with a zero block for the h-side n columns,
    which makes it the *same* device code path with different weights.
  - Biases are folded into the matmul via an extra contraction row of ones.
  - The break check "mean(h_k) > bc" latches the output at the first step k*
    whose global mean exceeds bc. The device free-runs L steps, records per-step
    per-partition sums (free side-output of the last fused DVE op), and the host
    computes the global means. If the break fires before the last step (it cannot
    for the harness inputs: |h|<1 and bc>=0.9 keeps means far below bc), the
    kernel is re-built with L=k* and re-run, which reproduces the latched output.

Device layout (per core, per step):
  - h is stored "packed": [128 partitions, 250 free] with partition 64*H+b
    holding h[b, 250*H + c]. All elementwise gate math runs on [128, 250] tiles.
  - Gate pre-activations go into two [128, 512] PSUM tiles: region 1 = [gr | gz]
    (packed: partition half H holds gate columns 250H:250H+250), region 2 =
    [gin | ghn]. Each region is computed by 4 K-chunk slots; each slot is a PAIR
    of concurrent column-tiled matmuls [K<=126, M=64, N=500]: tile A
    (tile_position (0,0)) produces PSUM partitions 0:64 (H=0 gate columns),
    tile B ((0,64)) partitions 64:128 (H=1 columns). The two tiles share the
    same stationary (h^T K-chunk, batch in array columns) and stream different
    weight blocks, so the pair streams in ~one matmul's time. This replaces the
    previous K-doubled M=128 scheme (2x redundant MACs).
  - The stationary h^T lives in a [126, 256] SBUF tile: K-chunk g at columns
    64g:64g+64, with h-dim blocks g0=0:125, g1=250:375, g2=125:250, g3=375:500
    (chosen so PE-transposing hnew[:, 0:125] yields chunks g0|g1 side by side
    and hnew[:, 125:250] yields g2|g3). Row 125 of chunk g1 holds the ones row
    for the bias fold (the matching weight blocks carry the biases).
  - The gate chain is split in two 125-column chunks; chunk 0's chain feeds
    transpose A early so the next step's first matmul pairs can start while
    chunk 1 is still in the pipes. SBUF-only elementwise ops (zm1, zh, t2) run
    on GPSIMD (no PSUM port there), PSUM-reading ops on DVE, activations and
    one transpose-copy per half on ACT.
"""

import os
import sys
import types
import numpy as np
import ml_dtypes

BF16NP = ml_dtypes.bfloat16

D = 500
B = 512
NCORES = 8
BS = B // NCORES          # 64 batch rows per core
HALF = 250                # free columns of the packed layout
# h-dim blocks of the 4 K-chunks (order matches the transpose outputs)
CHUNK_DIMS = ((0, 125), (250, 375), (125, 250), (375, 500))
KG = (125, 126, 125, 125)  # chunk g1 carries the ones/bias row


def _install_hook_module():
    """Provide antenv.axon_hooks (missing from the RO image) so NTFF tracing
    through bass_utils can work when requested. Harmless if anything fails."""
    if "antenv.axon_hooks" in sys.modules:
        return
    mod = types.ModuleType("antenv.axon_hooks")
    holder = [None]
    mod.set_axon_ntff_profile_hook = lambda h: holder.__setitem__(0, h)
    mod.get_axon_ntff_profile_hook = lambda: holder[0]
    sys.modules["antenv.axon_hooks"] = mod
    try:
        from trn_agent_boot.trn_boot import _ntff_profile_via_ctypes
        hook = _ntff_profile_via_ctypes("/opt/axon/libaxon_pjrt.so")
        mod.set_axon_ntff_profile_hook(hook)
    except Exception:
        pass


_install_hook_module()

import concourse.bass as bass  # noqa: E402
import concourse.mybir as mybir  # noqa: E402
import concourse.tile as tile  # noqa: E402
from concourse import bass_utils  # noqa: E402
from concourse.masks import make_identity  # noqa: E402
import bass_rust  # noqa: E402

F32 = mybir.dt.float32
F32R = mybir.dt.float32r
BF16 = mybir.dt.bfloat16
AF = mybir.ActivationFunctionType
ALU = mybir.AluOpType


def _split_overwide_waits(nc, maxw=1):
    """walrus here rejects >1 sync wait per instruction; spread extras over
    preceding NoOp carriers. Most multi-wait instructions get same-engine
    carriers (order-preserving); the kernel-end drain (many loose-end waits)
    gets carriers round-robined across all engines so they resolve in
    parallel before the final barrier instead of serially on one engine."""
    n_new = 0
    all_engines = (mybir.EngineType.SP, mybir.EngineType.Activation,
                   mybir.EngineType.PE, mybir.EngineType.DVE,
                   mybir.EngineType.Pool)
    for fn in nc.m.functions:
        for bb in fn.blocks:
            out = []
            for inst in bb.instructions:
                si = inst.sync_info
                if si is not None and si.on_wait and len(si.on_wait) > maxw:
                    waits = list(si.on_wait)
                    chunks = [waits[i:i + maxw] for i in range(0, len(waits), maxw)]
                    spread = len(chunks) > 4  # only the big end-of-kernel drain
                    for j, ch in enumerate(chunks[:-1]):
                        eng = all_engines[j % len(all_engines)] if spread \
                            else inst.engine
                        nd = mybir.InstNoOp(
                            name=f"I-swx{n_new}", engine=eng,
                            bass_nofuse=True,
                            sync_info=bass_rust.SyncInfo(on_wait=ch, on_update=[]))
                        n_new += 1
                        nc.register_instruction(nd, overwrite=True)
                        out.append(nd)
                    inst.sync_info = bass_rust.SyncInfo(
                        on_wait=chunks[-1], on_update=list(si.on_update or []))
                out.append(inst)
            bb.instructions = out
    return n_new


def _build(L):
    """Build the Bass module for L GRU steps. Returns nc."""
    assert L >= 1
    nc = bass.Bass("TRN2", target_bir_lowering=False, debug=False)

    statet_d = nc.dram_tensor("statet", [126, 256], BF16, kind="ExternalInput").ap()
    wa_d = nc.dram_tensor("wa", [2, 126, 4000], BF16, kind="ExternalInput").ap()
    wb_d = nc.dram_tensor("wb", [2, 126, 4000], BF16, kind="ExternalInput").ap()
    hout_d = nc.dram_tensor("hout", [128, HALF], F32R, kind="ExternalOutput").ap()
    sums_d = nc.dram_tensor("sums", [128, 2 * L], F32, kind="ExternalOutput").ap()

    with tile.TileContext(nc) as tc:
        import contextlib
        with contextlib.ExitStack() as ctx:
            consts = ctx.enter_context(tc.tile_pool(name="consts", bufs=1))
            wpool = ctx.enter_context(tc.tile_pool(name="weights", bufs=1))
            hpool = ctx.enter_context(tc.tile_pool(name="hstate", bufs=1))
            work = ctx.enter_context(tc.tile_pool(name="work", bufs=2))
            gpsum = ctx.enter_context(tc.tile_pool(name="gpsum", bufs=2, space="PSUM"))
            tpsum = ctx.enter_context(tc.tile_pool(name="tpsum", bufs=2, space="PSUM"))

            identity = consts.tile([128, 128], F32, tag="identity", name="identity")
            make_identity(nc, identity[:])
            # f32r view for the transposes (verifier wants an f32r producer)
            identity_r = consts.tile([128, 128], F32R, tag="identity_r", name="identity_r")
            nc.vector.tensor_copy(identity_r[:], identity[:])

            statet = wpool.tile([126, 256], BF16, tag="statet", name="statet")
            nc.gpsimd.dma_start(statet[:], statet_d)
            # fused weight loads: 4 large DMAs instead of 16 (the ~1-2 us
            # per-DMA issue overhead dominated kernel startup)
            wa_t = [wpool.tile([126, 4000], BF16, tag=f"wah{h}", name=f"wah{h}")
                    for h in range(2)]
            wb_t = [wpool.tile([126, 4000], BF16, tag=f"wbh{h}", name=f"wbh{h}")
                    for h in range(2)]
            nc.gpsimd.dma_start(wa_t[0][:], wa_d[0])
            nc.sync.dma_start(wa_t[1][:], wa_d[1])
            nc.gpsimd.dma_start(wb_t[0][:], wb_d[0])
            nc.sync.dma_start(wb_t[1][:], wb_d[1])

            hT = [hpool.tile([126, 256], BF16, tag="hta", name="hta"),
                  hpool.tile([126, 256], BF16, tag="htb", name="htb")]
            # only row 125 needs init (ones at chunk g1, zeros elsewhere);
            # rows 0:125 are fully overwritten by the transpose copies before
            # first use. DVE ops can't start at partition 125, DMA can.
            nc.gpsimd.dma_start(hT[0][125:126, :], statet_d[125:126, :])
            nc.gpsimd.dma_start(hT[1][125:126, :], statet_d[125:126, :])

            sums = consts.tile([128, 2 * L], F32, tag="sums", name="sums")

            hprev = None  # packed [128, 250] h of the previous step
            hnew = None
            for k in range(1, L + 1):
                first = k == 1
                lhs_tile = statet if first else hT[k % 2]
                wt = wa_t if first else wb_t

                # separate PSUM tiles per bank so the rz consumers don't
                # wait on the n-block matmuls
                grz = gpsum.tile([128, 512], F32, tag="grz", name="grz")
                gn = gpsum.tile([128, 512], F32, tag="gn", name="gn")

                def mm_slot(out_tile, col0, g, off_base, width):
                    kg = KG[g]
                    lhsT = lhs_tile[0:kg, 64 * g:64 * g + 64]
                    for H in (0, 1):
                        off = (g % 2) * 2000 + off_base + H * width
                        nc.tensor.matmul(
                            out_tile[64 * H:64 * H + 64, col0:col0 + width],
                            lhsT, wt[g // 2][0:kg, off:off + width],
                            start=(g == 0), stop=(g == 3),
                            tile_position=(0, 64 * H),
                            skip_group_check=True)

                # 12 pair-slots: rz g0-3 (N=500), then the n-region split in
                # column halves (N=250 each) so chunk 0's chain starts 4 slots
                # earlier than a monolithic n region would allow
                for g in range(4):
                    mm_slot(grz, 0, g, 0, 500)
                for g in range(4):
                    mm_slot(gn, 0, g, 1000, 250)
                for g in range(4):
                    mm_slot(gn, 250, g, 1500, 250)

                rz = work.tile([128, 2 * HALF], F32, tag="rz", name="rz")
                # r = sigmoid(gr); w = 1-z = sigmoid(-gz) straight from PSUM.
                # z itself is never materialized: z*h = h - w*h.
                nc.scalar.activation(rz[:, 0:250], grz[:, 0:250], AF.Sigmoid)
                r = rz[:, 0:250]
                zm1 = work.tile([128, HALF], F32, tag="zm1", name="zm1")
                nc.scalar.activation(zm1[:], grz[:, 250:500], AF.Sigmoid,
                                     scale=-1.0)

                rhn = work.tile([128, HALF], F32R, tag="rhn", name="rhn")
                targ = work.tile([128, HALF], F32R, tag="targ", name="targ")
                n = work.tile([128, HALF], F32R, tag="n", name="n")
                t2 = work.tile([128, HALF], F32R, tag="t2", name="t2")
                hnew = work.tile([128, HALF], F32R, tag="hnew", name="hnew")
                if k < L:
                    dst = hT[(k + 1) % 2]

                # PSUM-reading chain ops on DVE, chunk 0 first
                # (n-region PSUM layout is [gin0|ghn0|gin1|ghn1], 125 each)
                for c in (0, 1):
                    s = slice(125 * c, 125 * (c + 1))
                    nc.vector.tensor_mul(rhn[:, s], r[:, s],
                                         gn[:, 250 * c + 125:250 * c + 250])
                    nc.vector.tensor_add(targ[:, s], rhn[:, s],
                                         gn[:, 250 * c:250 * c + 125])
                nc.scalar.activation(n[:, 0:125], targ[:, 0:125], AF.Tanh)
                nc.scalar.activation(n[:, 125:250], targ[:, 125:250], AF.Tanh)

                # Dummy matmul chain through the serial tail: the HAM clock
                # gate re-throttles the PE to 1.2 GHz whenever an activity
                # window has significant idle time; once cold, the throttle
                # phase-locks against the matmul stream (stream cold, tail
                # warm). The dummies are UNANCHORED (ready at step start, so
                # zero start lag) and WAW-serialized through dmy, tiling the
                # tail back-to-back from the moment the stream ends. f32r
                # with N<256 runs at quarter rate -> ~420ns each at any clock.
                if k < L:
                    dmy = tpsum.tile([1, 512], F32, tag="dmy", name="dmy", bufs=1)

                def dummy(count=1):
                    # f32r M=1 dummies: ~115ns of PE-busy each at full clock,
                    # ~210ns cold, negligible power (one array column active;
                    # fp32-mode dummies here trip the chip-level P0 power
                    # throttle, downclocking every engine ~20%)
                    if k >= L or hprev is None:
                        return
                    for _ in range(count):
                        nc.tensor.matmul(dmy[:, 0:250], hprev[0:128, 0:1],
                                         hprev[0:128, 0:250])

                if k < L:
                    nc.tensor.matmul(dmy[:], lhs_tile[0:126, 0:1],
                                     wt[0][0:126, 0:512])
                    dummy(19)

                # zh = z*h = h - w*h as two pure-TensorTensor GPSIMD ops
                # (all-SBUF, off the critical path; Pool has no PSUM port and
                # no scalar-operand opcodes; keeps ACT at 2 sigmoids and DVE
                # at the chain)
                wh = work.tile([128, HALF], F32, tag="wh", name="wh")
                zh = work.tile([128, HALF], F32, tag="zh", name="zh")
                if not first:
                    # chunked so zh[:, 0:125] is ready before hnew chunk 0
                    for c in (0, 1):
                        s = slice(125 * c, 125 * (c + 1))
                        nc.gpsimd.tensor_mul(wh[:, s], zm1[:, s], hprev[:, s])
                        nc.gpsimd.tensor_sub(zh[:, s], hprev[:, s], wh[:, s])

                pT = [None, None]
                for c in (0, 1):
                    s = slice(125 * c, 125 * (c + 1))
                    acc = sums[:, 2 * (k - 1) + c:2 * (k - 1) + c + 1]
                    if first:
                        # h == 0: h_new = n * (1 - z)
                        nc.vector.scalar_tensor_tensor(
                            hnew[:, s], n[:, s], 0.0, zm1[:, s],
                            op0=ALU.bypass, op1=ALU.mult, accum_out=acc)
                    else:
                        nc.vector.tensor_mul(t2[:, s], n[:, s], zm1[:, s])
                        nc.vector.scalar_tensor_tensor(
                            hnew[:, s], t2[:, s], 0.0, zh[:, s],
                            op0=ALU.bypass, op1=ALU.add, accum_out=acc)
                    if k < L:
                        # transpose via a REGULAR matmul (in.T @ I): real PE
                        # activity for the HAM clock gate, and PSUM output
                        pT[c] = tpsum.tile([125, 128], F32, tag="pT",
                                           name=f"p{'AB'[c]}")
                        nc.tensor.matmul(pT[c][:], hnew[:, s], identity_r[:])
                        if c == 0:
                            dummy(2)
                if k < L:
                    # PSUM -> SBUF stationary copies: transpose A as one
                    # [125, 128] ACT copy; transpose B split ACT+DVE halves
                    # (its consumers are the last stream slots)
                    nc.scalar.copy(dst[0:125, 0:128], pT[0][:])
                    nc.scalar.copy(dst[0:125, 128:192], pT[1][:, 0:64])
                    nc.vector.tensor_copy(dst[0:125, 192:256], pT[1][:, 64:128])
                hprev = hnew

            nc.gpsimd.dma_start(hout_d, hnew[:])
            nc.gpsimd.dma_start(sums_d, sums[:])

    _split_overwide_waits(nc)
    return nc


_NC_CACHE = {}


def _get_nc(L):
    if L not in _NC_CACHE:
        _NC_CACHE[L] = _build(L)
    return _NC_CACHE[L]


def _prep_weights(W_ih, W_hh, b_ih, b_hh):
    """Build wa/wb DRAM images [2, 126, 4000].

    Column layout per K-chunk g (base (g%2)*2000):
      +0    : RZ block,  H0: [r(250)|z(250)]      +500  : H1 same
      +1000 : N0 block,  H0: [gin-c0(125)|ghn-c0(125)]   +1250: H1
      +1500 : N1 block,  H0: [gin-c1|ghn-c1]             +1750: H1
    Rows 0:125 = W^T rows for the chunk's h-dims; row 125 = bias
    (nonzero only for chunk g1)."""
    W_ih = np.asarray(W_ih, np.float32)
    W_hh = np.asarray(W_hh, np.float32)
    b_ih = np.asarray(b_ih, np.float32)
    b_hh = np.asarray(b_hh, np.float32)

    br = b_ih[0:500] + b_hh[0:500]
    bz = b_ih[500:1000] + b_hh[500:1000]
    bin_ = b_ih[1000:1500]
    bhn = b_hh[1000:1500]

    def put(img, g, off, left, right, bias):
        w = left.shape[0]
        img[g // 2, 0:125, off:off + 2 * w] = np.hstack([left.T, right.T])
        if g == 1:
            img[g // 2, 125, off:off + 2 * w] = bias

    def pack(Wrz, Win, Whn):
        img = np.zeros((2, 126, 4000), np.float32)
        for g in range(4):
            d0, d1 = CHUNK_DIMS[g]
            base = (g % 2) * 2000
            for H in range(2):
                lo = 250 * H
                # RZ block (500 wide per half)
                put(img, g, base + 500 * H,
                    Wrz[lo:lo + 250, d0:d1], Wrz[500 + lo:500 + lo + 250, d0:d1],
                    np.concatenate([br[lo:lo + 250], bz[lo:lo + 250]]))
                # N0 / N1 blocks (250 wide per half, chain-chunk column split)
                for cb in range(2):
                    cl = lo + 125 * cb
                    put(img, g, base + 1000 + 500 * cb + 250 * H,
                        Win[cl:cl + 125, d0:d1], Whn[cl:cl + 125, d0:d1],
                        np.concatenate([bin_[cl:cl + 125], bhn[cl:cl + 125]]))
        return np.ascontiguousarray(img.astype(BF16NP))

    zeros = np.zeros((500, D), np.float32)
    WB = pack(W_ih[0:1000] + W_hh[0:1000], W_ih[1000:1500], W_hh[1000:1500])
    WA = pack(W_ih[0:1000], W_ih[1000:1500], zeros)
    return WA, WB


def _prep_state(state):
    """Per-core stationary state^T images [126, 256] (chunk g at cols 64g)."""
    state = np.asarray(state, np.float32)
    outs = []
    for c in range(NCORES):
        shard = state[BS * c:BS * (c + 1)]            # [64, 500]
        img = np.zeros((126, 256), np.float32)
        for g in range(4):
            d0, d1 = CHUNK_DIMS[g]
            img[0:125, 64 * g:64 * g + 64] = shard[:, d0:d1].T
        img[125, 64:128] = 1.0                        # ones row of chunk g1
        outs.append(img.astype(BF16NP))
    return outs


def _run(L, stateTs, wa, wb, trace=False):
    nc = _get_nc(L)
    in_maps = [{"statet": np.ascontiguousarray(stateTs[c]),
                "wa": wa, "wb": wb} for c in range(NCORES)]
    res = bass_utils.run_bass_kernel_spmd(
        nc, in_maps, core_ids=list(range(NCORES)), trace=trace)
    shards = []
    sums = np.zeros((128, 2 * L), np.float64)
    for c in range(NCORES):
        hout = res.results[c]["hout"]
        shards.append(np.concatenate([hout[0:64], hout[64:128]], axis=1))
        sums += res.results[c]["sums"].astype(np.float64)
    h = np.concatenate(shards, axis=0)                # [512, 500]
    means = (sums[:, 0::2] + sums[:, 1::2]).sum(axis=0) / (B * D)  # [L]
    return h, means, res


def kernel(state, W_ih, W_hh, b_ih, b_hh, break_condition, recursion_limit):
    state = np.asarray(state, np.float32)
    L = int(np.asarray(recursion_limit))
    if L <= 0:
        return state.copy()
    bc = float(np.asarray(break_condition))

    wa, wb = _prep_weights(W_ih, W_hh, b_ih, b_hh)
    stateTs = _prep_state(state)

    h, means, _ = _run(L, stateTs, wa, wb)
    fired = np.nonzero(means > bc)[0]
    if fired.size and fired[0] + 1 < L:
        # break fired at step k* = fired[0]+1: output latches h_{k*}
        h, _, _ = _run(int(fired[0]) + 1, stateTs, wa, wb)
    return h.astype(np.float32)


# revision 43
# speedup vs baseline: 1.1896x; 1.0026x over previous
"""Trainium2 Bass kernel for nn_EternalRecursion (GRUCell self-recursion, B=512, D=500).

Strategy
--------
Data-parallel over 8 NeuronCores: 64 batch rows per core, GRU weights replicated.

Math restructuring (host-side, exact):
  - After step 1 the reference feeds h_new as BOTH x and h of the GRU cell, so
    steps >= 2 use combined weights W_rz = (W_ih+W_hh)[0:1000] for the r/z gates,
    while the n-gate keeps W_ih_n / W_hh_n separate (r multiplies only the h-side).
  - Step 1 (x=state, h=0) uses W_ih with a zero block for the h-side n columns,
    which makes it the *same* device code path with different weights.
  - Biases are folded into the matmul via an extra contraction row of ones.
  - The break check "mean(h_k) > bc" latches the output at the first step k*
    whose global mean exceeds bc. The device free-runs L steps, records per-step
    per-partition sums (free side-output of the last fused DVE op), and the host
    computes the global means. If the break fires before the last step (it cannot
    for the harness inputs: |h|<1 and bc>=0.9 keeps means far below bc), the
    kernel is re-built with L=k* and re-run, which reproduces the latched output.

Device layout (per core, per step):
  - h is stored "packed": [128 partitions, 250 free] with partition 64*H+b
    holding h[b, 250*H + c]. All elementwise gate math runs on [128, 250] tiles.
  - Gate pre-activations go into two [128, 512] PSUM tiles: region 1 = [gr | gz]
    (packed: partition half H holds gate columns 250H:250H+250), region 2 =
    [gin0|ghn0|gin1|ghn1] (the n-region is split in two column halves so the
    chunk-0 chain starts 4 slots early). Each region block is computed by 4
    K-chunk slots; each slot is a PAIR of concurrent column-tiled matmuls
    [K<=126, M=64]: tile A (tile_position (0,0)) produces PSUM partitions 0:64
    (H=0 gate columns), tile B ((0,64)) partitions 64:128 (H=1 columns). The
    two tiles share the same stationary (h^T K-chunk, batch in array columns)
    and stream different weight blocks, so the pair streams in ~one matmul's
    time. This replaces the previous K-doubled M=128 scheme (2x redundant
    MACs). bf16 operands (f32 PSUM accumulation): f32r forbids dst partition
    base 64, and bf16 keeps rel err at 3.2e-3 against the 2e-2 budget.
  - The stationary h^T lives in a [126, 256] SBUF tile: K-chunk g at columns
    64g:64g+64, with h-dim blocks g0=0:125, g1=250:375, g2=125:250, g3=375:500
    (chosen so PE-transposing hnew[:, 0:125] yields chunks g0|g1 side by side
    and hnew[:, 125:250] yields g2|g3). Row 125 of chunk g1 holds the ones row
    for the bias fold (the matching weight blocks carry the biases).
  - The gate chain is split in two 125-column chunks; chunk 0's chain feeds
    transpose A early so the next step's first matmul pairs can start while
    chunk 1 is still in the pipes. z is never materialized: w = 1-z =
    sigmoid(-gz) via the ACT scale, z*h = h - w*h (two pure-TensorTensor
    GPSIMD ops; the Pool engine has no PSUM port and no scalar-operand
    opcodes). PSUM-reading chain ops and the hnew combines run on DVE,
    sigmoids/tanhs and most transpose copies on ACT.
  - HAM clock-gate management turned out to be the dominant effect: with a
    ~2us warm stream and a ~2.5us serial tail per step, the PE activity
    monitor phase-locks K=4/8 (1.2 GHz) onto the stream and K=8/8 onto the
    idle tail, halving matmul throughput. A WAW-serialized chain of M=1 f32r
    dummy matmuls (~115ns each, negligible power) tiles the tail back-to-back
    so every activity window looks busy; full coverage is reproducibly worth
    ~70us total. fp32-mode dummies (and under-coverage) instead trip a
    chip-level downclock that slows ACT/DVE/GPSIMD by ~20%.
"""

import os
import sys
import types
import numpy as np
import ml_dtypes

BF16NP = ml_dtypes.bfloat16

D = 500
B = 512
NCORES = 8
BS = B // NCORES          # 64 batch rows per core
HALF = 250                # free columns of the packed layout
# h-dim blocks of the 4 K-chunks (order matches the transpose outputs)
CHUNK_DIMS = ((0, 125), (250, 375), (125, 250), (375, 500))
KG = (125, 126, 125, 125)  # chunk g1 carries the ones/bias row


def _install_hook_module():
    """Provide antenv.axon_hooks (missing from the RO image) so NTFF tracing
    through bass_utils can work when requested. Harmless if anything fails."""
    if "antenv.axon_hooks" in sys.modules:
        return
    mod = types.ModuleType("antenv.axon_hooks")
    holder = [None]
    mod.set_axon_ntff_profile_hook = lambda h: holder.__setitem__(0, h)
    mod.get_axon_ntff_profile_hook = lambda: holder[0]
    sys.modules["antenv.axon_hooks"] = mod
    try:
        from trn_agent_boot.trn_boot import _ntff_profile_via_ctypes
        hook = _ntff_profile_via_ctypes("/opt/axon/libaxon_pjrt.so")
        mod.set_axon_ntff_profile_hook(hook)
    except Exception:
        pass


_install_hook_module()

import concourse.bass as bass  # noqa: E402
import concourse.mybir as mybir  # noqa: E402
import concourse.tile as tile  # noqa: E402
from concourse import bass_utils  # noqa: E402
from concourse.masks import make_identity  # noqa: E402
import bass_rust  # noqa: E402

F32 = mybir.dt.float32
F32R = mybir.dt.float32r
BF16 = mybir.dt.bfloat16
AF = mybir.ActivationFunctionType
ALU = mybir.AluOpType


def _split_overwide_waits(nc, maxw=1):
    """walrus here rejects >1 sync wait per instruction; spread extras over
    preceding NoOp carriers. Most multi-wait instructions get same-engine
    carriers (order-preserving); the kernel-end drain (many loose-end waits)
    gets carriers round-robined across all engines so they resolve in
    parallel before the final barrier instead of serially on one engine."""
    n_new = 0
    all_engines = (mybir.EngineType.SP, mybir.EngineType.Activation,
                   mybir.EngineType.PE, mybir.EngineType.DVE,
                   mybir.EngineType.Pool)
    for fn in nc.m.functions:
        for bb in fn.blocks:
            out = []
            for inst in bb.instructions:
                si = inst.sync_info
                if si is not None and si.on_wait and len(si.on_wait) > maxw:
                    waits = list(si.on_wait)
                    chunks = [waits[i:i + maxw] for i in range(0, len(waits), maxw)]
                    spread = len(chunks) > 4  # only the big end-of-kernel drain
                    for j, ch in enumerate(chunks[:-1]):
                        eng = all_engines[j % len(all_engines)] if spread \
                            else inst.engine
                        nd = mybir.InstNoOp(
                            name=f"I-swx{n_new}", engine=eng,
                            bass_nofuse=True,
                            sync_info=bass_rust.SyncInfo(on_wait=ch, on_update=[]))
                        n_new += 1
                        nc.register_instruction(nd, overwrite=True)
                        out.append(nd)
                    inst.sync_info = bass_rust.SyncInfo(
                        on_wait=chunks[-1], on_update=list(si.on_update or []))
                out.append(inst)
            bb.instructions = out
    return n_new


def _build(L):
    """Build the Bass module for L GRU steps. Returns nc."""
    assert L >= 1
    nc = bass.Bass("TRN2", target_bir_lowering=False, debug=False)

    statet_d = nc.dram_tensor("statet", [126, 256], BF16, kind="ExternalInput").ap()
    wa_d = nc.dram_tensor("wa", [2, 126, 4000], BF16, kind="ExternalInput").ap()
    wb_d = nc.dram_tensor("wb", [2, 126, 4000], BF16, kind="ExternalInput").ap()
    hout_d = nc.dram_tensor("hout", [128, HALF], F32R, kind="ExternalOutput").ap()
    sums_d = nc.dram_tensor("sums", [128, 2 * L], F32, kind="ExternalOutput").ap()

    with tile.TileContext(nc) as tc:
        import contextlib
        with contextlib.ExitStack() as ctx:
            consts = ctx.enter_context(tc.tile_pool(name="consts", bufs=1))
            wpool = ctx.enter_context(tc.tile_pool(name="weights", bufs=1))
            hpool = ctx.enter_context(tc.tile_pool(name="hstate", bufs=1))
            work = ctx.enter_context(tc.tile_pool(name="work", bufs=2))
            gpsum = ctx.enter_context(tc.tile_pool(name="gpsum", bufs=2, space="PSUM"))
            tpsum = ctx.enter_context(tc.tile_pool(name="tpsum", bufs=2, space="PSUM"))

            identity = consts.tile([128, 128], F32, tag="identity", name="identity")
            make_identity(nc, identity[:])
            # f32r view for the transposes (verifier wants an f32r producer)
            identity_r = consts.tile([128, 128], F32R, tag="identity_r", name="identity_r")
            nc.vector.tensor_copy(identity_r[:], identity[:])

            statet = wpool.tile([126, 256], BF16, tag="statet", name="statet")
            nc.gpsimd.dma_start(statet[:], statet_d)
            # fused weight loads: 4 large DMAs instead of 16 (the ~1-2 us
            # per-DMA issue overhead dominated kernel startup)
            wa_t = [wpool.tile([126, 4000], BF16, tag=f"wah{h}", name=f"wah{h}")
                    for h in range(2)]
            wb_t = [wpool.tile([126, 4000], BF16, tag=f"wbh{h}", name=f"wbh{h}")
                    for h in range(2)]
            nc.gpsimd.dma_start(wa_t[0][:], wa_d[0])
            nc.sync.dma_start(wa_t[1][:], wa_d[1])
            nc.gpsimd.dma_start(wb_t[0][:], wb_d[0])
            nc.sync.dma_start(wb_t[1][:], wb_d[1])

            hT = [hpool.tile([126, 256], BF16, tag="hta", name="hta"),
                  hpool.tile([126, 256], BF16, tag="htb", name="htb")]
            # only row 125 needs init (ones at chunk g1, zeros elsewhere);
            # rows 0:125 are fully overwritten by the transpose copies before
            # first use. DVE ops can't start at partition 125, DMA can.
            nc.gpsimd.dma_start(hT[0][125:126, :], statet_d[125:126, :])
            nc.gpsimd.dma_start(hT[1][125:126, :], statet_d[125:126, :])

            sums = consts.tile([128, 2 * L], F32, tag="sums", name="sums")

            hprev = None  # packed [128, 250] h of the previous step
            hnew = None
            for k in range(1, L + 1):
                first = k == 1
                lhs_tile = statet if first else hT[k % 2]
                wt = wa_t if first else wb_t

                # separate PSUM tiles per bank so the rz consumers don't
                # wait on the n-block matmuls
                grz = gpsum.tile([128, 512], F32, tag="grz", name="grz")
                gn = gpsum.tile([128, 512], F32, tag="gn", name="gn")

                def mm_slot(out_tile, col0, g, off_base, width):
                    kg = KG[g]
                    lhsT = lhs_tile[0:kg, 64 * g:64 * g + 64]
                    for H in (0, 1):
                        off = (g % 2) * 2000 + off_base + H * width
                        nc.tensor.matmul(
                            out_tile[64 * H:64 * H + 64, col0:col0 + width],
                            lhsT, wt[g // 2][0:kg, off:off + width],
                            start=(g == 0), stop=(g == 3),
                            tile_position=(0, 64 * H),
                            skip_group_check=True)

                # 12 pair-slots: rz g0-3 (N=500), then the n-region split in
                # column halves (N=250 each) so chunk 0's chain starts 4 slots
                # earlier than a monolithic n region would allow
                for g in range(4):
                    mm_slot(grz, 0, g, 0, 500)
                for g in range(4):
                    mm_slot(gn, 0, g, 1000, 250)
                for g in range(4):
                    mm_slot(gn, 250, g, 1500, 250)

                rz = work.tile([128, 2 * HALF], F32, tag="rz", name="rz")
                # r = sigmoid(gr); w = 1-z = sigmoid(-gz) straight from PSUM.
                # z itself is never materialized: z*h = h - w*h.
                nc.scalar.activation(rz[:, 0:250], grz[:, 0:250], AF.Sigmoid)
                r = rz[:, 0:250]
                zm1 = work.tile([128, HALF], F32, tag="zm1", name="zm1")
                nc.scalar.activation(zm1[:], grz[:, 250:500], AF.Sigmoid,
                                     scale=-1.0)

                rhn = work.tile([128, HALF], F32R, tag="rhn", name="rhn")
                targ = work.tile([128, HALF], F32R, tag="targ", name="targ")
                n = work.tile([128, HALF], F32R, tag="n", name="n")
                t2 = work.tile([128, HALF], F32R, tag="t2", name="t2")
                hnew = work.tile([128, HALF], F32R, tag="hnew", name="hnew")
                if k < L:
                    dst = hT[(k + 1) % 2]

                # PSUM-reading chain ops on DVE, chunk 0 first
                # (n-region PSUM layout is [gin0|ghn0|gin1|ghn1], 125 each)
                for c in (0, 1):
                    s = slice(125 * c, 125 * (c + 1))
                    nc.vector.tensor_mul(rhn[:, s], r[:, s],
                                         gn[:, 250 * c + 125:250 * c + 250])
                    nc.vector.tensor_add(targ[:, s], rhn[:, s],
                                         gn[:, 250 * c:250 * c + 125])
                nc.scalar.activation(n[:, 0:125], targ[:, 0:125], AF.Tanh)
                nc.scalar.activation(n[:, 125:250], targ[:, 125:250], AF.Tanh)

                # Dummy matmul chain through the serial tail: the HAM clock
                # gate re-throttles the PE to 1.2 GHz whenever an activity
                # window has significant idle time; once cold, the throttle
                # phase-locks against the matmul stream (stream cold, tail
                # warm). The dummies are UNANCHORED (ready at step start, so
                # zero start lag) and WAW-serialized through dmy, tiling the
                # tail back-to-back from the moment the stream ends. f32r
                # with N<256 runs at quarter rate -> ~420ns each at any clock.
                if k < L:
                    dmy = tpsum.tile([1, 512], F32, tag="dmy", name="dmy", bufs=1)

                def dummy(count=1):
                    # f32r M=1 dummies: ~115ns of PE-busy each at full clock,
                    # ~210ns cold, negligible power (one array column active;
                    # fp32-mode dummies here trip the chip-level P0 power
                    # throttle, downclocking every engine ~20%)
                    if k >= L or hprev is None:
                        return
                    for _ in range(count):
                        nc.tensor.matmul(dmy[:, 0:250], hprev[0:128, 0:1],
                                         hprev[0:128, 0:250])

                if k < L:
                    nc.tensor.matmul(dmy[:], lhs_tile[0:126, 0:1],
                                     wt[0][0:126, 0:512])
                    dummy(19)

                # zh = z*h = h - w*h as two pure-TensorTensor GPSIMD ops
                # (all-SBUF, off the critical path; Pool has no PSUM port and
                # no scalar-operand opcodes; keeps ACT at 2 sigmoids and DVE
                # at the chain)
                wh = work.tile([128, HALF], F32, tag="wh", name="wh")
                zh = work.tile([128, HALF], F32, tag="zh", name="zh")
                if not first:
                    # chunked so zh[:, 0:125] is ready before hnew chunk 0
                    for c in (0, 1):
                        s = slice(125 * c, 125 * (c + 1))
                        nc.gpsimd.tensor_mul(wh[:, s], zm1[:, s], hprev[:, s])
                        nc.gpsimd.tensor_sub(zh[:, s], hprev[:, s], wh[:, s])

                pT = [None, None]
                for c in (0, 1):
                    s = slice(125 * c, 125 * (c + 1))
                    acc = sums[:, 2 * (k - 1) + c:2 * (k - 1) + c + 1]
                    if first:
                        # h == 0: h_new = n * (1 - z)
                        nc.vector.scalar_tensor_tensor(
                            hnew[:, s], n[:, s], 0.0, zm1[:, s],
                            op0=ALU.bypass, op1=ALU.mult, accum_out=acc)
                    else:
                        nc.vector.tensor_mul(t2[:, s], n[:, s], zm1[:, s])
                        nc.vector.scalar_tensor_tensor(
                            hnew[:, s], t2[:, s], 0.0, zh[:, s],
                            op0=ALU.bypass, op1=ALU.add, accum_out=acc)
                    if k < L:
                        # transpose via a REGULAR matmul (in.T @ I): real PE
                        # activity for the HAM clock gate, and PSUM output
                        pT[c] = tpsum.tile([125, 128], F32, tag="pT",
                                           name=f"p{'AB'[c]}")
                        nc.tensor.matmul(pT[c][:], hnew[:, s], identity_r[:])
                        if c == 0:
                            dummy(2)
                if k < L:
                    # PSUM -> SBUF stationary copies: transpose A as one
                    # [125, 128] ACT copy; transpose B split ACT+DVE halves
                    # (its consumers are the last stream slots)
                    nc.scalar.copy(dst[0:125, 0:128], pT[0][:])
                    nc.scalar.copy(dst[0:125, 128:192], pT[1][:, 0:64])
                    nc.vector.tensor_copy(dst[0:125, 192:256], pT[1][:, 64:128])
                hprev = hnew

            nc.gpsimd.dma_start(hout_d, hnew[:])
            nc.gpsimd.dma_start(sums_d, sums[:])

    _split_overwide_waits(nc)
    return nc


_NC_CACHE = {}


def _get_nc(L):
    if L not in _NC_CACHE:
        _NC_CACHE[L] = _build(L)
    return _NC_CACHE[L]


def _prep_weights(W_ih, W_hh, b_ih, b_hh):
    """Build wa/wb DRAM images [2, 126, 4000].

    Column layout per K-chunk g (base (g%2)*2000):
      +0    : RZ block,  H0: [r(250)|z(250)]      +500  : H1 same
      +1000 : N0 block,  H0: [gin-c0(125)|ghn-c0(125)]   +1250: H1
      +1500 : N1 block,  H0: [gin-c1|ghn-c1]             +1750: H1
    Rows 0:125 = W^T rows for the chunk's h-dims; row 125 = bias
    (nonzero only for chunk g1)."""
    W_ih = np.asarray(W_ih, np.float32)
    W_hh = np.asarray(W_hh, np.float32)
    b_ih = np.asarray(b_ih, np.float32)
    b_hh = np.asarray(b_hh, np.float32)

    br = b_ih[0:500] + b_hh[0:500]
    bz = b_ih[500:1000] + b_hh[500:1000]
    bin_ = b_ih[1000:1500]
    bhn = b_hh[1000:1500]

    def put(img, g, off, left, right, bias):
        w = left.shape[0]
        img[g // 2, 0:125, off:off + 2 * w] = np.hstack([left.T, right.T])
        if g == 1:
            img[g // 2, 125, off:off + 2 * w] = bias

    def pack(Wrz, Win, Whn):
        img = np.zeros((2, 126, 4000), np.float32)
        for g in range(4):
            d0, d1 = CHUNK_DIMS[g]
            base = (g % 2) * 2000
            for H in range(2):
                lo = 250 * H
                # RZ block (500 wide per half)
                put(img, g, base + 500 * H,
                    Wrz[lo:lo + 250, d0:d1], Wrz[500 + lo:500 + lo + 250, d0:d1],
                    np.concatenate([br[lo:lo + 250], bz[lo:lo + 250]]))
                # N0 / N1 blocks (250 wide per half, chain-chunk column split)
                for cb in range(2):
                    cl = lo + 125 * cb
                    put(img, g, base + 1000 + 500 * cb + 250 * H,
                        Win[cl:cl + 125, d0:d1], Whn[cl:cl + 125, d0:d1],
                        np.concatenate([bin_[cl:cl + 125], bhn[cl:cl + 125]]))
        return np.ascontiguousarray(img.astype(BF16NP))

    zeros = np.zeros((500, D), np.float32)
    WB = pack(W_ih[0:1000] + W_hh[0:1000], W_ih[1000:1500], W_hh[1000:1500])
    WA = pack(W_ih[0:1000], W_ih[1000:1500], zeros)
    return WA, WB


def _prep_state(state):
    """Per-core stationary state^T images [126, 256] (chunk g at cols 64g)."""
    state = np.asarray(state, np.float32)
    outs = []
    for c in range(NCORES):
        shard = state[BS * c:BS * (c + 1)]            # [64, 500]
        img = np.zeros((126, 256), np.float32)
        for g in range(4):
            d0, d1 = CHUNK_DIMS[g]
            img[0:125, 64 * g:64 * g + 64] = shard[:, d0:d1].T
        img[125, 64:128] = 1.0                        # ones row of chunk g1
        outs.append(img.astype(BF16NP))
    return outs


def _run(L, stateTs, wa, wb, trace=False):
    nc = _get_nc(L)
    in_maps = [{"statet": np.ascontiguousarray(stateTs[c]),
                "wa": wa, "wb": wb} for c in range(NCORES)]
    res = bass_utils.run_bass_kernel_spmd(
        nc, in_maps, core_ids=list(range(NCORES)), trace=trace)
    shards = []
    sums = np.zeros((128, 2 * L), np.float64)
    for c in range(NCORES):
        hout = res.results[c]["hout"]
        shards.append(np.concatenate([hout[0:64], hout[64:128]], axis=1))
        sums += res.results[c]["sums"].astype(np.float64)
    h = np.concatenate(shards, axis=0)                # [512, 500]
    means = (sums[:, 0::2] + sums[:, 1::2]).sum(axis=0) / (B * D)  # [L]
    return h, means, res


def kernel(state, W_ih, W_hh, b_ih, b_hh, break_condition, recursion_limit):
    state = np.asarray(state, np.float32)
    L = int(np.asarray(recursion_limit))
    if L <= 0:
        return state.copy()
    bc = float(np.asarray(break_condition))

    wa, wb = _prep_weights(W_ih, W_hh, b_ih, b_hh)
    stateTs = _prep_state(state)

    h, means, _ = _run(L, stateTs, wa, wb)
    fired = np.nonzero(means > bc)[0]
    if fired.size and fired[0] + 1 < L:
        # break fired at step k* = fired[0]+1: output latches h_{k*}
        h, _, _ = _run(int(fired[0]) + 1, stateTs, wa, wb)
    return h.astype(np.float32)


# revision 45
# speedup vs baseline: 1.1985x; 1.0074x over previous
"""Trainium2 Bass kernel for nn_EternalRecursion (GRUCell self-recursion, B=512, D=500).

Strategy
--------
Data-parallel over 8 NeuronCores: 64 batch rows per core, GRU weights replicated.

Math restructuring (host-side, exact):
  - After step 1 the reference feeds h_new as BOTH x and h of the GRU cell, so
    steps >= 2 use combined weights W_rz = (W_ih+W_hh)[0:1000] for the r/z gates,
    while the n-gate keeps W_ih_n / W_hh_n separate (r multiplies only the h-side).
  - Step 1 (x=state, h=0) uses W_ih with a zero block for the h-side n columns,
    which makes it the *same* device code path with different weights.
  - Biases are folded into the matmul via an extra contraction row of ones.
  - The break check "mean(h_k) > bc" latches the output at the first step k*
    whose global mean exceeds bc. The device free-runs L steps, records per-step
    per-partition sums (free side-output of the last fused DVE op), and the host
    computes the global means. If the break fires before the last step (it cannot
    for the harness inputs: |h|<1 and bc>=0.9 keeps means far below bc), the
    kernel is re-built with L=k* and re-run, which reproduces the latched output.

Device layout (per core, per step):
  - h is stored "packed": [128 partitions, 250 free] with partition 64*H+b
    holding h[b, 250*H + c]. All elementwise gate math runs on [128, 250] tiles.
  - Gate pre-activations go into two [128, 512] PSUM tiles: region 1 = [gr | gz]
    (packed: partition half H holds gate columns 250H:250H+250), region 2 =
    [gin0|ghn0|gin1|ghn1] (the n-region is split in two column halves so the
    chunk-0 chain starts 4 slots early). Each region block is computed by 4
    K-chunk slots; each slot is a PAIR of concurrent column-tiled matmuls
    [K<=126, M=64]: tile A (tile_position (0,0)) produces PSUM partitions 0:64
    (H=0 gate columns), tile B ((0,64)) partitions 64:128 (H=1 columns). The
    two tiles share the same stationary (h^T K-chunk, batch in array columns)
    and stream different weight blocks, so the pair streams in ~one matmul's
    time. This replaces the previous K-doubled M=128 scheme (2x redundant
    MACs). bf16 operands (f32 PSUM accumulation): f32r forbids dst partition
    base 64, and bf16 keeps rel err at 3.2e-3 against the 2e-2 budget.
  - The stationary h^T lives in a [126, 256] SBUF tile: K-chunk g at columns
    64g:64g+64, with h-dim blocks g0=0:125, g1=250:375, g2=125:250, g3=375:500
    (chosen so PE-transposing hnew[:, 0:125] yields chunks g0|g1 side by side
    and hnew[:, 125:250] yields g2|g3). Row 125 of chunk g1 holds the ones row
    for the bias fold (the matching weight blocks carry the biases).
  - The gate chain is split in two 125-column chunks; chunk 0's chain feeds
    transpose A early so the next step's first matmul pairs can start while
    chunk 1 is still in the pipes. z is never materialized: w = 1-z =
    sigmoid(-gz) via the ACT scale, z*h = h - w*h (two pure-TensorTensor
    GPSIMD ops; the Pool engine has no PSUM port and no scalar-operand
    opcodes). PSUM-reading chain ops and the hnew combines run on DVE,
    sigmoids/tanhs and most transpose copies on ACT.
  - HAM clock-gate management turned out to be the dominant effect: with a
    ~2us warm stream and a ~2.5us serial tail per step, the PE activity
    monitor phase-locks K=4/8 (1.2 GHz) onto the stream and K=8/8 onto the
    idle tail, halving matmul throughput. A WAW-serialized chain of M=1 f32r
    dummy matmuls (~115ns each, negligible power) tiles the tail back-to-back
    so every activity window looks busy; full coverage is reproducibly worth
    ~70us total. fp32-mode dummies (and under-coverage) instead trip a
    chip-level downclock that slows ACT/DVE/GPSIMD by ~20%.
"""

import os
import sys
import types
import numpy as np
import ml_dtypes

BF16NP = ml_dtypes.bfloat16

D = 500
B = 512
NCORES = 8
BS = B // NCORES          # 64 batch rows per core
HALF = 250                # free columns of the packed layout
# h-dim blocks of the 4 K-chunks (order matches the transpose outputs)
CHUNK_DIMS = ((0, 125), (250, 375), (125, 250), (375, 500))
KG = (125, 126, 125, 125)  # chunk g1 carries the ones/bias row


def _install_hook_module():
    """Provide antenv.axon_hooks (missing from the RO image) so NTFF tracing
    through bass_utils can work when requested. Harmless if anything fails."""
    if "antenv.axon_hooks" in sys.modules:
        return
    mod = types.ModuleType("antenv.axon_hooks")
    holder = [None]
    mod.set_axon_ntff_profile_hook = lambda h: holder.__setitem__(0, h)
    mod.get_axon_ntff_profile_hook = lambda: holder[0]
    sys.modules["antenv.axon_hooks"] = mod
    try:
        from trn_agent_boot.trn_boot import _ntff_profile_via_ctypes
        hook = _ntff_profile_via_ctypes("/opt/axon/libaxon_pjrt.so")
        mod.set_axon_ntff_profile_hook(hook)
    except Exception:
        pass


_install_hook_module()

import concourse.bass as bass  # noqa: E402
import concourse.mybir as mybir  # noqa: E402
import concourse.tile as tile  # noqa: E402
from concourse import bass_utils  # noqa: E402
from concourse.masks import make_identity  # noqa: E402
import bass_rust  # noqa: E402

F32 = mybir.dt.float32
F32R = mybir.dt.float32r
BF16 = mybir.dt.bfloat16
AF = mybir.ActivationFunctionType
ALU = mybir.AluOpType


def _split_overwide_waits(nc, maxw=1):
    """walrus here rejects >1 sync wait per instruction; spread extras over
    preceding NoOp carriers. Most multi-wait instructions get same-engine
    carriers (order-preserving); the kernel-end drain (many loose-end waits)
    gets carriers round-robined across all engines so they resolve in
    parallel before the final barrier instead of serially on one engine."""
    n_new = 0
    all_engines = (mybir.EngineType.SP, mybir.EngineType.Activation,
                   mybir.EngineType.PE, mybir.EngineType.DVE,
                   mybir.EngineType.Pool)
    for fn in nc.m.functions:
        for bb in fn.blocks:
            out = []
            for inst in bb.instructions:
                si = inst.sync_info
                if si is not None and si.on_wait and len(si.on_wait) > maxw:
                    waits = list(si.on_wait)
                    chunks = [waits[i:i + maxw] for i in range(0, len(waits), maxw)]
                    spread = len(chunks) > 4  # only the big end-of-kernel drain
                    for j, ch in enumerate(chunks[:-1]):
                        eng = all_engines[j % len(all_engines)] if spread \
                            else inst.engine
                        nd = mybir.InstNoOp(
                            name=f"I-swx{n_new}", engine=eng,
                            bass_nofuse=True,
                            sync_info=bass_rust.SyncInfo(on_wait=ch, on_update=[]))
                        n_new += 1
                        nc.register_instruction(nd, overwrite=True)
                        out.append(nd)
                    inst.sync_info = bass_rust.SyncInfo(
                        on_wait=chunks[-1], on_update=list(si.on_update or []))
                out.append(inst)
            bb.instructions = out
    return n_new


def _build(L):
    """Build the Bass module for L GRU steps. Returns nc."""
    assert L >= 1
    nc = bass.Bass("TRN2", target_bir_lowering=False, debug=False)

    statet_d = nc.dram_tensor("statet", [126, 256], BF16, kind="ExternalInput").ap()
    wa_d = nc.dram_tensor("wa", [2, 126, 4000], BF16, kind="ExternalInput").ap()
    wb_d = nc.dram_tensor("wb", [2, 126, 4000], BF16, kind="ExternalInput").ap()
    hout_d = nc.dram_tensor("hout", [128, HALF], F32R, kind="ExternalOutput").ap()
    sums_d = nc.dram_tensor("sums", [128, 2 * L], F32, kind="ExternalOutput").ap()

    with tile.TileContext(nc) as tc:
        import contextlib
        with contextlib.ExitStack() as ctx:
            consts = ctx.enter_context(tc.tile_pool(name="consts", bufs=1))
            wpool = ctx.enter_context(tc.tile_pool(name="weights", bufs=1))
            hpool = ctx.enter_context(tc.tile_pool(name="hstate", bufs=1))
            work = ctx.enter_context(tc.tile_pool(name="work", bufs=2))
            gpsum = ctx.enter_context(tc.tile_pool(name="gpsum", bufs=2, space="PSUM"))
            tpsum = ctx.enter_context(tc.tile_pool(name="tpsum", bufs=2, space="PSUM"))

            identity = consts.tile([128, 128], F32, tag="identity", name="identity")
            make_identity(nc, identity[:])
            # f32r view for the transposes (verifier wants an f32r producer)
            identity_r = consts.tile([128, 128], F32R, tag="identity_r", name="identity_r")
            nc.vector.tensor_copy(identity_r[:], identity[:])

            statet = wpool.tile([126, 256], BF16, tag="statet", name="statet")
            nc.gpsimd.dma_start(statet[:], statet_d)
            # fused weight loads: 4 large DMAs instead of 16 (the ~1-2 us
            # per-DMA issue overhead dominated kernel startup)
            wa_t = [wpool.tile([126, 4000], BF16, tag=f"wah{h}", name=f"wah{h}")
                    for h in range(2)]
            wb_t = [wpool.tile([126, 4000], BF16, tag=f"wbh{h}", name=f"wbh{h}")
                    for h in range(2)]
            nc.gpsimd.dma_start(wa_t[0][:], wa_d[0])
            nc.sync.dma_start(wa_t[1][:], wa_d[1])
            nc.gpsimd.dma_start(wb_t[0][:], wb_d[0])
            nc.sync.dma_start(wb_t[1][:], wb_d[1])

            hT = [hpool.tile([126, 256], BF16, tag="hta", name="hta"),
                  hpool.tile([126, 256], BF16, tag="htb", name="htb")]
            # only row 125 needs init (ones at chunk g1, zeros elsewhere);
            # rows 0:125 are fully overwritten by the transpose copies before
            # first use. DVE ops can't start at partition 125, DMA can.
            nc.gpsimd.dma_start(hT[0][125:126, :], statet_d[125:126, :])
            nc.gpsimd.dma_start(hT[1][125:126, :], statet_d[125:126, :])

            sums = consts.tile([128, 2 * L], F32, tag="sums", name="sums")

            hprev = None  # packed [128, 250] h of the previous step
            hnew = None
            for k in range(1, L + 1):
                first = k == 1
                lhs_tile = statet if first else hT[k % 2]
                wt = wa_t if first else wb_t

                # separate PSUM tiles per bank so the rz consumers don't
                # wait on the n-block matmuls
                grz = gpsum.tile([128, 512], F32, tag="grz", name="grz")
                gn = gpsum.tile([128, 512], F32, tag="gn", name="gn")

                def mm_slot(out_tile, col0, g, off_base, width):
                    kg = KG[g]
                    lhsT = lhs_tile[0:kg, 64 * g:64 * g + 64]
                    for H in (0, 1):
                        off = (g % 2) * 2000 + off_base + H * width
                        nc.tensor.matmul(
                            out_tile[64 * H:64 * H + 64, col0:col0 + width],
                            lhsT, wt[g // 2][0:kg, off:off + width],
                            start=(g == 0), stop=(g == 3),
                            tile_position=(0, 64 * H),
                            skip_group_check=True)

                # 12 pair-slots: rz g0-3 (N=500), then the n-region split in
                # column halves (N=250 each) so chunk 0's chain starts 4 slots
                # earlier than a monolithic n region would allow
                for g in range(4):
                    mm_slot(grz, 0, g, 0, 500)
                for g in range(4):
                    mm_slot(gn, 0, g, 1000, 250)
                for g in range(4):
                    mm_slot(gn, 250, g, 1500, 250)

                rz = work.tile([128, 2 * HALF], F32, tag="rz", name="rz")
                # r = sigmoid(gr); w = 1-z = sigmoid(-gz) straight from PSUM.
                # z itself is never materialized: z*h = h - w*h.
                nc.scalar.activation(rz[:, 0:250], grz[:, 0:250], AF.Sigmoid)
                r = rz[:, 0:250]
                zm1 = work.tile([128, HALF], F32, tag="zm1", name="zm1")
                nc.scalar.activation(zm1[:], grz[:, 250:500], AF.Sigmoid,
                                     scale=-1.0)

                rhn = work.tile([128, HALF], F32R, tag="rhn", name="rhn")
                targ = work.tile([128, HALF], F32R, tag="targ", name="targ")
                n = work.tile([128, HALF], F32R, tag="n", name="n")
                t2 = work.tile([128, HALF], F32R, tag="t2", name="t2")
                hnew = work.tile([128, HALF], F32R, tag="hnew", name="hnew")
                if k < L:
                    dst = hT[(k + 1) % 2]

                # PSUM-reading chain ops on DVE, chunk 0 first
                # (n-region PSUM layout is [gin0|ghn0|gin1|ghn1], 125 each)
                for c in (0, 1):
                    s = slice(125 * c, 125 * (c + 1))
                    nc.vector.tensor_mul(rhn[:, s], r[:, s],
                                         gn[:, 250 * c + 125:250 * c + 250])
                    nc.vector.tensor_add(targ[:, s], rhn[:, s],
                                         gn[:, 250 * c:250 * c + 125])
                nc.scalar.activation(n[:, 0:125], targ[:, 0:125], AF.Tanh)
                nc.scalar.activation(n[:, 125:250], targ[:, 125:250], AF.Tanh)

                # Dummy matmul chain through the serial tail: the HAM clock
                # gate re-throttles the PE to 1.2 GHz whenever an activity
                # window has significant idle time; once cold, the throttle
                # phase-locks against the matmul stream (stream cold, tail
                # warm). The dummies are UNANCHORED (ready at step start, so
                # zero start lag) and WAW-serialized through dmy, tiling the
                # tail back-to-back from the moment the stream ends. f32r
                # with N<256 runs at quarter rate -> ~420ns each at any clock.
                if k < L:
                    dmy = tpsum.tile([1, 512], F32, tag="dmy", name="dmy", bufs=1)

                def dummy(count=1):
                    # f32r M=1 dummies: ~115ns of PE-busy each at full clock,
                    # ~210ns cold, negligible power (one array column active;
                    # fp32-mode dummies here trip the chip-level P0 power
                    # throttle, downclocking every engine ~20%)
                    if k >= L or hprev is None:
                        return
                    for _ in range(count):
                        nc.tensor.matmul(dmy[:, 0:250], hprev[0:128, 0:1],
                                         hprev[0:128, 0:250])

                if k < L:
                    nc.tensor.matmul(dmy[:], lhs_tile[0:126, 0:1],
                                     wt[0][0:126, 0:512])
                    dummy(19)

                # zh = z*h = h - w*h as two pure-TensorTensor GPSIMD ops
                # (all-SBUF, off the critical path; Pool has no PSUM port and
                # no scalar-operand opcodes; keeps ACT at 2 sigmoids and DVE
                # at the chain)
                wh = work.tile([128, HALF], F32R, tag="wh", name="wh")
                zh = work.tile([128, HALF], F32R, tag="zh", name="zh")
                if not first:
                    # chunked so zh[:, 0:125] is ready before hnew chunk 0
                    for c in (0, 1):
                        s = slice(125 * c, 125 * (c + 1))
                        nc.gpsimd.tensor_mul(wh[:, s], zm1[:, s], hprev[:, s])
                        nc.gpsimd.tensor_sub(zh[:, s], hprev[:, s], wh[:, s])

                pT = [None, None]
                for c in (0, 1):
                    s = slice(125 * c, 125 * (c + 1))
                    if first:
                        # h == 0: h_new = n * (1 - z)
                        nc.vector.scalar_tensor_tensor(
                            hnew[:, s], n[:, s], 0.0, zm1[:, s],
                            op0=ALU.bypass, op1=ALU.mult,
                            accum_out=sums[:, 2 * (k - 1) + c:2 * (k - 1) + c + 1])
                    else:
                        nc.vector.tensor_mul(t2[:, s], n[:, s], zm1[:, s])
                    if k < L:
                        # next step's stationary = hnew^T, computed DIRECTLY
                        # in PSUM as zh^T + t2^T (two accumulating transpose
                        # matmuls): the elementwise hnew combine drops off the
                        # stationary critical path entirely. The zh transpose
                        # runs early (zh is ready mid-stream from GPSIMD).
                        pT[c] = tpsum.tile([125, 128], F32, tag="pT",
                                           name=f"p{'AB'[c]}")
                        if first:
                            nc.tensor.matmul(pT[c][:], hnew[:, s], identity_r[:])
                        else:
                            nc.tensor.matmul(pT[c][:], zh[:, s], identity_r[:],
                                             start=True, stop=False)
                            nc.tensor.matmul(pT[c][:], t2[:, s], identity_r[:],
                                             start=False, stop=True)
                        if c == 0:
                            dummy(2)
                if not first:
                    # off-path: h_new = t2 + zh (feeds hout, sums, next wh/zh)
                    for c in (0, 1):
                        nc.vector.scalar_tensor_tensor(
                            hnew[:, 125 * c:125 * (c + 1)],
                            t2[:, 125 * c:125 * (c + 1)],
                            0.0, zh[:, 125 * c:125 * (c + 1)],
                            op0=ALU.bypass, op1=ALU.add,
                            accum_out=sums[:, 2 * (k - 1) + c:2 * (k - 1) + c + 1])
                if k < L:
                    # PSUM -> SBUF stationary copies: transpose A as one
                    # [125, 128] ACT copy; transpose B split ACT+DVE halves
                    # (its consumers are the last stream slots)
                    nc.scalar.copy(dst[0:125, 0:128], pT[0][:])
                    nc.scalar.copy(dst[0:125, 128:192], pT[1][:, 0:64])
                    nc.vector.tensor_copy(dst[0:125, 192:256], pT[1][:, 64:128])
                hprev = hnew

            nc.gpsimd.dma_start(hout_d, hnew[:])
            nc.gpsimd.dma_start(sums_d, sums[:])

    _split_overwide_waits(nc)
    return nc


_NC_CACHE = {}


def _get_nc(L):
    if L not in _NC_CACHE:
        _NC_CACHE[L] = _build(L)
    return _NC_CACHE[L]


def _prep_weights(W_ih, W_hh, b_ih, b_hh):
    """Build wa/wb DRAM images [2, 126, 4000].

    Column layout per K-chunk g (base (g%2)*2000):
      +0    : RZ block,  H0: [r(250)|z(250)]      +500  : H1 same
      +1000 : N0 block,  H0: [gin-c0(125)|ghn-c0(125)]   +1250: H1
      +1500 : N1 block,  H0: [gin-c1|ghn-c1]             +1750: H1
    Rows 0:125 = W^T rows for the chunk's h-dims; row 125 = bias
    (nonzero only for chunk g1)."""
    W_ih = np.asarray(W_ih, np.float32)
    W_hh = np.asarray(W_hh, np.float32)
    b_ih = np.asarray(b_ih, np.float32)
    b_hh = np.asarray(b_hh, np.float32)

    br = b_ih[0:500] + b_hh[0:500]
    bz = b_ih[500:1000] + b_hh[500:1000]
    bin_ = b_ih[1000:1500]
    bhn = b_hh[1000:1500]

    def put(img, g, off, left, right, bias):
        w = left.shape[0]
        img[g // 2, 0:125, off:off + 2 * w] = np.hstack([left.T, right.T])
        if g == 1:
            img[g // 2, 125, off:off + 2 * w] = bias

    def pack(Wrz, Win, Whn):
        img = np.zeros((2, 126, 4000), np.float32)
        for g in range(4):
            d0, d1 = CHUNK_DIMS[g]
            base = (g % 2) * 2000
            for H in range(2):
                lo = 250 * H
                # RZ block (500 wide per half)
                put(img, g, base + 500 * H,
                    Wrz[lo:lo + 250, d0:d1], Wrz[500 + lo:500 + lo + 250, d0:d1],
                    np.concatenate([br[lo:lo + 250], bz[lo:lo + 250]]))
                # N0 / N1 blocks (250 wide per half, chain-chunk column split)
                for cb in range(2):
                    cl = lo + 125 * cb
                    put(img, g, base + 1000 + 500 * cb + 250 * H,
                        Win[cl:cl + 125, d0:d1], Whn[cl:cl + 125, d0:d1],
                        np.concatenate([bin_[cl:cl + 125], bhn[cl:cl + 125]]))
        return np.ascontiguousarray(img.astype(BF16NP))

    zeros = np.zeros((500, D), np.float32)
    WB = pack(W_ih[0:1000] + W_hh[0:1000], W_ih[1000:1500], W_hh[1000:1500])
    WA = pack(W_ih[0:1000], W_ih[1000:1500], zeros)
    return WA, WB


def _prep_state(state):
    """Per-core stationary state^T images [126, 256] (chunk g at cols 64g)."""
    state = np.asarray(state, np.float32)
    outs = []
    for c in range(NCORES):
        shard = state[BS * c:BS * (c + 1)]            # [64, 500]
        img = np.zeros((126, 256), np.float32)
        for g in range(4):
            d0, d1 = CHUNK_DIMS[g]
            img[0:125, 64 * g:64 * g + 64] = shard[:, d0:d1].T
        img[125, 64:128] = 1.0                        # ones row of chunk g1
        outs.append(img.astype(BF16NP))
    return outs


def _run(L, stateTs, wa, wb, trace=False):
    nc = _get_nc(L)
    in_maps = [{"statet": np.ascontiguousarray(stateTs[c]),
                "wa": wa, "wb": wb} for c in range(NCORES)]
    res = bass_utils.run_bass_kernel_spmd(
        nc, in_maps, core_ids=list(range(NCORES)), trace=trace)
    shards = []
    sums = np.zeros((128, 2 * L), np.float64)
    for c in range(NCORES):
        hout = res.results[c]["hout"]
        shards.append(np.concatenate([hout[0:64], hout[64:128]], axis=1))
        sums += res.results[c]["sums"].astype(np.float64)
    h = np.concatenate(shards, axis=0)                # [512, 500]
    means = (sums[:, 0::2] + sums[:, 1::2]).sum(axis=0) / (B * D)  # [L]
    return h, means, res


def kernel(state, W_ih, W_hh, b_ih, b_hh, break_condition, recursion_limit):
    state = np.asarray(state, np.float32)
    L = int(np.asarray(recursion_limit))
    if L <= 0:
        return state.copy()
    bc = float(np.asarray(break_condition))

    wa, wb = _prep_weights(W_ih, W_hh, b_ih, b_hh)
    stateTs = _prep_state(state)

    h, means, _ = _run(L, stateTs, wa, wb)
    fired = np.nonzero(means > bc)[0]
    if fired.size and fired[0] + 1 < L:
        # break fired at step k* = fired[0]+1: output latches h_{k*}
        h, _, _ = _run(int(fired[0]) + 1, stateTs, wa, wb)
    return h.astype(np.float32)
